# revision 2
# baseline (speedup 1.0000x reference)
"""Trainium2 8-core causal single-head attention.

Problem: x[4,4096,768] @ Wq/Wk/Wv[768,64] -> causal softmax attention -> out[4,4096,64].

Sharding: 8 cores = 4 batches x 2 query-interleave groups. Core c handles
batch b=c//2, parity h=c%2: local q-tile i (16 tiles of 128 rows) maps to
global q-tile g=2i+h. Both cores of a batch compute full-context K/V
projections locally (no collectives). Causal block structure is
SPMD-uniform: local q-tile i attends kv-tiles 0..2i+1, with the top two
kv tiles masked by per-core data masks (h=0: [tri, zero]; h=1: [ones, tri]).

On-chip layout: everything transposed. Host passes xT (so DMA is contiguous
and projections contract e on partitions). Scores are computed as
S^T[kv_p, q_f] = kT_tile.T @ qT so the exp output PT is directly the lhsT...
actually the *moving* operand of PV: outT[d1, q] += v1[kv,65].T @ PT[kv, q],
with v1 = [v | ones] so row 64 of outT accumulates the softmax denominator.
No max subtraction (scores ~N(0,1), |s|<~6) and no on-chip reductions at all.
Host divides by the denominator and scatters tiles back.
"""

import sys

sys.path.insert(0, "/opt/trn_rl_repo")

from contextlib import ExitStack

import numpy as np
import ml_dtypes

B, T, E, D = 4, 4096, 768, 64
P = 128
TQ = T // 2          # queries per core
NQT = TQ // P        # 16 local q tiles
NKV = T // P         # 32 kv tiles
EC = E // P          # 6 contraction chunks
BF16 = ml_dtypes.bfloat16

_CACHE = {}


def _build_bass():
    import concourse.bacc as bacc
    import concourse.mybir as mybir
    import concourse.tile as tile
    from concourse import bass

    nc = bacc.Bacc("TRN2", target_bir_lowering=False)
    f32 = mybir.dt.float32
    bf16 = mybir.dt.bfloat16

    xkv_d = nc.dram_tensor("xkv", (E, T), bf16, kind="ExternalInput")
    xq_d = nc.dram_tensor("xq", (E, TQ), bf16, kind="ExternalInput")
    wq_d = nc.dram_tensor("wq", (E, D), bf16, kind="ExternalInput")
    wkv_d = nc.dram_tensor("wkv", (E, 2 * D), bf16, kind="ExternalInput")
    mprev_d = nc.dram_tensor("mask_prev", (P, P), bf16, kind="ExternalInput")
    mlast_d = nc.dram_tensor("mask_last", (P, P), bf16, kind="ExternalInput")
    ident_d = nc.dram_tensor("ident", (P, D), bf16, kind="ExternalInput")
    out_d = nc.dram_tensor("out", (D + 1, TQ), f32, kind="ExternalOutput")

    with ExitStack() as ctx:
        tc = ctx.enter_context(tile.TileContext(nc))
        const = ctx.enter_context(tc.tile_pool(name="const", bufs=1))
        xpool = ctx.enter_context(tc.tile_pool(name="x", bufs=1))
        spool = ctx.enter_context(tc.tile_pool(name="sb", bufs=1))
        ptpool = ctx.enter_context(tc.tile_pool(name="pt", bufs=3))
        obpool = ctx.enter_context(tc.tile_pool(name="ob", bufs=2))
        pproj = ctx.enter_context(tc.tile_pool(name="pproj", bufs=2, space="PSUM"))
        ps_s = ctx.enter_context(tc.tile_pool(name="ps", bufs=2, space="PSUM"))
        pout = ctx.enter_context(tc.tile_pool(name="pout", bufs=1, space="PSUM"))

        # ---- constants ----
        wq_t = const.tile([P, EC * D], bf16)
        nc.sync.dma_start(
            out=wq_t.rearrange("p (ec d) -> p ec d", d=D),
            in_=wq_d.rearrange("(ec p) d -> p ec d", p=P),
        )
        wkv_t = const.tile([P, EC * 2 * D], bf16)
        nc.sync.dma_start(
            out=wkv_t.rearrange("p (ec d) -> p ec d", d=2 * D),
            in_=wkv_d.rearrange("(ec p) d -> p ec d", p=P),
        )
        mprev_t = const.tile([P, P], bf16)
        nc.sync.dma_start(out=mprev_t[:], in_=mprev_d[:])
        mlast_t = const.tile([P, P], bf16)
        nc.sync.dma_start(out=mlast_t[:], in_=mlast_d[:])
        ident_t = const.tile([P, D], bf16)
        nc.sync.dma_start(out=ident_t[:], in_=ident_d[:])

        # ---- input DMA (chunked for overlap) ----
        xq_t = xpool.tile([P, EC * TQ], bf16)
        for j in range(TQ // 512):
            nc.sync.dma_start(
                out=xq_t.rearrange("p (ec t) -> p ec t", t=TQ)[:, :, j * 512:(j + 1) * 512],
                in_=xq_d.rearrange("(ec p) t -> p ec t", p=P)[:, :, j * 512:(j + 1) * 512],
            )
        xkv_t = xpool.tile([P, EC * T], bf16)
        for j in range(T // 512):
            nc.sync.dma_start(
                out=xkv_t.rearrange("p (ec t) -> p ec t", t=T)[:, :, j * 512:(j + 1) * 512],
                in_=xkv_d.rearrange("(ec p) t -> p ec t", p=P)[:, :, j * 512:(j + 1) * 512],
            )

        # ---- qT projection: qT[d, tq] ----
        qT_t = spool.tile([D, TQ], bf16)
        for j in range(TQ // 512):
            ps = pproj.tile([P, 512], mybir.dt.float32, tag="pp")
            for ec in range(EC):
                nc.tensor.matmul(
                    ps[0:D, :],
                    lhsT=wq_t[:, ec * D:(ec + 1) * D],
                    rhs=xq_t[:, ec * TQ + j * 512: ec * TQ + (j + 1) * 512],
                    start=(ec == 0),
                    stop=(ec == EC - 1),
                )
            nc.vector.tensor_copy(qT_t[:, j * 512:(j + 1) * 512], ps[0:D, :])

        # ---- k/v projection (packed): kvT rows 0..63 = kT, 64..127 = vT ----
        kvT_t = spool.tile([P, T], bf16)
        for j in range(T // 512):
            ps = pproj.tile([P, 512], mybir.dt.float32, tag="pp")
            for ec in range(EC):
                nc.tensor.matmul(
                    ps,
                    lhsT=wkv_t[:, ec * 2 * D:(ec + 1) * 2 * D],
                    rhs=xkv_t[:, ec * T + j * 512: ec * T + (j + 1) * 512],
                    start=(ec == 0),
                    stop=(ec == EC - 1),
                )
            nc.vector.tensor_copy(kvT_t[:, j * 512:(j + 1) * 512], ps)

        # ---- v1 tiles: v1[:, k*65:k*65+64] = v rows of kv tile k; col 64 = 1 ----
        v1_t = spool.tile([P, NKV * (D + 1)], bf16)
        nc.vector.memset(v1_t[:], 1.0)
        for k in range(NKV):
            pv = pproj.tile([P, D], bf16, tag="pp")
            nc.tensor.transpose(
                pv[:], in_=kvT_t[D:2 * D, k * P:(k + 1) * P], identity=ident_t[D:2 * D, :]
            )
            nc.vector.tensor_copy(v1_t[:, k * (D + 1): k * (D + 1) + D], pv[:])

        # ---- attention: for kv tile k, S^T strip over q cols [128*(k//2) .. 2048) ----
        outp = pout.tile([D + 1, TQ], mybir.dt.float32)
        for k in range(NKV):
            qs = (k // 2) * P
            j0 = qs // 512
            for c in range(j0, TQ // 512):
                cs = max(qs, c * 512)
                ce = (c + 1) * 512
                w = ce - cs
                sst = ps_s.tile([P, 512], mybir.dt.float32)
                nc.tensor.matmul(
                    sst[:, 0:w],
                    lhsT=kvT_t[0:D, k * P:(k + 1) * P],
                    rhs=qT_t[:, cs:ce],
                    start=True,
                    stop=True,
                )
                pt = ptpool.tile([P, 512], bf16)
                nc.scalar.activation(
                    pt[:, 0:w], sst[:, 0:w],
                    func=mybir.ActivationFunctionType.Exp, scale=0.125,
                )
                if c == j0:
                    m = mprev_t if (k % 2 == 0) else mlast_t
                    nc.vector.tensor_mul(pt[:, 0:P], pt[:, 0:P], m[:])
                nc.tensor.matmul(
                    outp[:, cs:ce],
                    lhsT=v1_t[:, k * (D + 1):(k + 1) * (D + 1)],
                    rhs=pt[:, 0:w],
                    start=(k == 0),
                    stop=(k == 8 * c + 7),
                )

        # ---- drain output ----
        for c in range(TQ // 512):
            ob = obpool.tile([D + 1, 512], mybir.dt.float32)
            nc.vector.tensor_copy(ob[:], outp[:, c * 512:(c + 1) * 512])
            nc.sync.dma_start(out=out_d[:, c * 512:(c + 1) * 512], in_=ob[:])

    nc.compile()
    return nc


def _shard_inputs(x, Wq, Wk, Wv):
    x = np.asarray(x, np.float32)
    wqb = np.asarray(Wq, np.float32).astype(BF16)
    wkvb = np.concatenate([np.asarray(Wk, np.float32), np.asarray(Wv, np.float32)], axis=1).astype(BF16)
    ident = np.zeros((P, D), BF16)
    ident[D:2 * D, :] = np.eye(D, dtype=BF16)
    tri = (np.arange(P)[:, None] <= np.arange(P)[None, :]).astype(BF16)
    ones = np.ones((P, P), BF16)
    zeros = np.zeros((P, P), BF16)
    qidx = {h: np.concatenate([np.arange(P) + (2 * i + h) * P for i in range(NQT)]) for h in (0, 1)}
    in_maps = []
    for c in range(8):
        b, h = c // 2, c % 2
        xT = np.ascontiguousarray(x[b].T).astype(BF16)      # [768, 4096]
        xq = np.ascontiguousarray(xT[:, qidx[h]])           # [768, 2048]
        in_maps.append({
            "xkv": xT,
            "xq": xq,
            "wq": wqb,
            "wkv": wkvb,
            "mask_prev": tri if h == 0 else ones,
            "mask_last": zeros if h == 0 else tri,
            "ident": ident,
        })
    return in_maps


def _unshard(results):
    out = np.zeros((B, T, D), np.float32)
    for c, om in enumerate(results):
        b, h = c // 2, c % 2
        o = np.asarray(om["out"], np.float32)               # [65, 2048]
        on = (o[:D] / o[D:D + 1]).T                         # [2048, 64]
        for i in range(NQT):
            out[b, (2 * i + h) * P:(2 * i + h + 1) * P] = on[i * P:(i + 1) * P]
    return out


def kernel(x, Wq, Wk, Wv):
    from concourse import bass_utils

    if "nc" not in _CACHE:
        _CACHE["nc"] = _build_bass()
    nc = _CACHE["nc"]
    in_maps = _shard_inputs(x, Wq, Wk, Wv)
    res = bass_utils.run_bass_kernel_spmd(nc, in_maps, core_ids=list(range(8)))
    _CACHE["last_result"] = res
    return _unshard(res.results)
